# revision 10
# baseline (speedup 1.0000x reference)
"""ColBERT intra-batch MaxSim scoring kernel for 8 Trainium2 NeuronCores.

Math (see reference):
  Q = l2norm(q_hidden @ W.T)                       [B, LQ, DIM]
  D = l2norm(d_hidden @ W.T); D masked             [B, LD, DIM]
  sim[b,c,q,k] = Q[b,q]·D[c,k]; masked k -> -inf
  out[b,c] = sum_q max_k sim

Sharding: docs (dim c) are sharded 16-per-core; q_hidden/W replicated.
Each core computes its [B, 16] slice of the score matrix.

Design notes (v2):
  * The DVE reduce_max over the sim matrix is the hard floor: every sim
    element must pass through the vector engine once at ~1 elem/cycle/lane
    (~81us/core for 4096 q-tokens x 2432 doc-token cols / 128 lanes); no
    other engine can reduce along the free axis from PSUM. The kernel is
    therefore organized so DVE runs reduces back-to-back and everything
    else hides under them.
  * ALL activations/weights move and multiply in bf16 (fp32 PE matmuls are
    ~3.5x slower and fp32 DMA is 2x the bytes); PSUM accumulation stays
    fp32. Host casts inputs to bf16 after transposing.
  * Host pre-transposes activations to [HID, tokens] so every matmul has
    its contraction dim on partitions. The doc mask is folded away on the
    host: valid tokens gathered front, tail padded with dup of the first
    valid token (dups never change a max) -> no masking on device.
  * Q is NOT normalized before the sim matmul: max_k is invariant under a
    positive per-query scale, so 1/|Q| is folded into the block-ones
    lhsT of the final query-sum matmul.
  * D IS normalized before the sim matmul (1/|d_k| does not commute with
    max_k): ones-matmul sumsq -> sqrt (ACT) + fast reciprocal (DVE) ->
    K=1 ones outer-product matmul broadcasts 1/|D| to 128 partitions ->
    DVE multiply straight out of the projection PSUM into bf16 SBUF.
  * DVE reads at most ONE PSUM operand per instruction (HW rule), and
    only ~1 elem/cycle in every mode, so no fold tricks help; the plain
    grouped reduce (one per sim half-tile) is optimal.
"""

import os

import numpy as np

B, LQ, LD, HID, DIM = 128, 32, 256, 768, 128
NCORES = 8
DPC = B // NCORES          # docs per core
TQ = B * LQ                # total query tokens
KC = HID // 128            # contraction chunks for the projection


def _chunks(total, step):
    """[(off, len)] cut at `step` boundaries — a matmul's PSUM output must
    stay inside a single 512-float bank, so chunks may never straddle one."""
    return [(o, min(step, total - o)) for o in range(0, total, step)]


def _build_program(NV_A, NV_B):
    import concourse.bass as bass  # noqa: F401
    import concourse.tile as tile
    from concourse import bacc, mybir

    f32 = mybir.dt.float32
    bf16 = mybir.dt.bfloat16
    AF = mybir.ActivationFunctionType
    AX = mybir.AxisListType
    ALU = mybir.AluOpType

    # two doc classes: the 8 longest docs (padded to NV_A) in half A, the 8
    # shortest (padded to NV_B <= NV_A) in half B — same instruction count,
    # ~8% fewer sim/reduce elements than uniform padding
    NVH = [(DPC // 2) * NV_A, (DPC // 2) * NV_B]
    HB = [0, NVH[0]]               # half base offsets
    NVS = [NV_A, NV_B]
    NVT = NVH[0] + NVH[1]          # compacted doc tokens per core
    NQCH = TQ // 512        # q-projection column chunks
    NTT = TQ // 128         # sim lhsT tiles (query-token tiles)
    BPT = 128 // LQ         # batch entries per query-token tile
    QG = 1024               # qt DMA column-group width

    nc = bacc.Bacc(
        "TRN2",
        target_bir_lowering=False,
        debug=False,
        num_devices=NCORES,
    )

    qT_d = nc.dram_tensor("qT", [HID, TQ], bf16, kind="ExternalInput")
    dT_d = nc.dram_tensor("dT", [HID, NVT], bf16, kind="ExternalInput")
    wT_d = nc.dram_tensor("wT", [128, KC, DIM], bf16, kind="ExternalInput")
    qso_d = nc.dram_tensor("qso", [128, BPT], f32, kind="ExternalInput")
    onescol_d = nc.dram_tensor("onescol", [128, 1], bf16, kind="ExternalInput")
    onesrow_d = nc.dram_tensor("onesrow", [1, 128], bf16, kind="ExternalInput")
    out_d = nc.dram_tensor("out", [B, DPC], f32, kind="ExternalOutput")

    with tile.TileContext(nc) as tc, tc.tile_pool(name="persist", bufs=1) as per:
        # --- constants + persistent SBUF tensors ---------------------------
        wt = per.tile([128, KC, DIM], bf16, name="wt")
        qso = per.tile([128, BPT], f32, name="qso")
        onescol = per.tile([128, 1], bf16, name="onescol")
        onesrow = per.tile([1, 128], bf16, name="onesrow")
        QT = per.tile([128, TQ], bf16, name="QT")         # q-proj [d, t] unnormalized
        DTn = per.tile([128, NVT], bf16, name="DTn")      # normalized d-proj
        invnQ = per.tile([128, NTT], f32, name="invnQ")   # 1/|Q| per query token
        normQ = per.tile([128, NTT], f32, name="normQ")
        lhsQ = per.tile([128, NTT, BPT], f32, name="lhsQ")  # blockones * 1/|Q|
        ssqD_row = per.tile([1, NVT], f32, name="ssqD_row")
        invnD32 = per.tile([1, NVT], f32, name="invnD32")
        invnD_row = per.tile([1, NVT], bf16, name="invnD_row")
        rowtmp = per.tile([1, NVT], f32, name="rowtmp")
        outstage = per.tile([BPT, NTT * DPC], f32, name="outstage")

        # constants go first on the gpsimd queue so wt is resident before
        # the first projection matmul; dT halves ride two queues in parallel
        # (sync: half A, gpsimd: half B) so phase D is DMA-gated for only
        # ~2MB per queue; qt jg0 rides the otherwise-idle scalar queue
        nc.gpsimd.dma_start(wt[:], wT_d[:, :, :])
        nc.gpsimd.dma_start(qso[:], qso_d[:, :])
        nc.gpsimd.dma_start(onescol[:], onescol_d[:, :])
        nc.gpsimd.dma_start(onesrow[:], onesrow_d[:, :])

        # ---------------- phase D: project + normalize doc tokens ---------
        # dT halves land on the sync queue (nothing else competes there);
        # qt column groups land on the gpsimd queue. Scalar/vector issue no
        # DMAs — their cycles belong to copies and reduces.
        qs_stack = tc.tile_pool(name="qt_pool", bufs=1)
        qt_pool = qs_stack.__enter__()
        qts = {}

        def load_jg(jg, eng=None):
            for k in range(KC):
                t_ = qt_pool.tile(
                    [128, QG], bf16, name=f"qt{k}_{jg}", tag=f"qt{k}",
                    bufs=2,
                )
                (eng or nc.gpsimd).dma_start(
                    t_[:], qT_d[k * 128:(k + 1) * 128, jg * QG:(jg + 1) * QG]
                )
                qts[(k, jg)] = t_

        # NOTE: scalar may only issue a FEW upfront DMAs — a back-pressured
        # issue blocks its in-order queue and stalls the phase-D squares
        # (measured +25us on the ramp with 12 queued issues).

        with (
            tc.tile_pool(name="dt_pool", bufs=1) as dt_pool,
            tc.tile_pool(name="psD", bufs=1, space="PSUM") as psD,
            tc.tile_pool(name="ssD", bufs=1, space="PSUM") as ssD,
            tc.tile_pool(name="sqD_pool", bufs=2) as sqD_pool,
            tc.tile_pool(name="psB", bufs=1, space="PSUM") as psB,
        ):
            # dT rides BOTH the sync and gpsimd queues (k-parity interleave,
            # half A first) so phase D is DMA-gated for ~1.7MB per queue; the
            # first two qt column groups ride the vector queue — the DVE is
            # idle until the first reduce anyway
            dts = {}
            for h in range(2):
                for k in range(KC):
                    dtk = dt_pool.tile(
                        [128, NVH[h]], bf16, name=f"dt{k}_{h}", tag=f"dt{k}_{h}"
                    )
                    eng = nc.sync if k % 2 == 0 else nc.gpsimd
                    eng.dma_start(
                        dtk[:],
                        dT_d[k * 128:(k + 1) * 128, HB[h]:HB[h] + NVH[h]],
                    )
                    dts[(k, h)] = dtk
            load_jg(0, eng=nc.scalar)
            load_jg(1)

            for h in range(2):
                base = HB[h]
                h_chunks = _chunks(NVH[h], 512)
                psd = psD.tile([128, NVH[h]], f32, name="psd", tag=f"psd{h}")
                for k in range(KC):
                    for (off, ln) in h_chunks:
                        nc.tensor.matmul(
                            psd[:, off:off + ln],
                            wt[:, k, :],
                            dts[(k, h)][:, off:off + ln],
                            start=(k == 0),
                            stop=(k == KC - 1),
                        )
                for (off, ln) in h_chunks:
                    sl = slice(off, off + ln)
                    gsl = slice(base + off, base + off + ln)
                    sq = sqD_pool.tile([128, 512], bf16, name="sqd", tag="sq")
                    nc.scalar.activation(sq[:, :ln], psd[:, sl], AF.Square)
                    ssd = ssD.tile([1, 512], f32, name="ssd", tag="ssd")
                    nc.tensor.matmul(
                        ssd[:, :ln], onescol[:], sq[:, :ln], start=True, stop=True
                    )
                    nc.scalar.copy(ssqD_row[:, gsl], ssd[:, :ln])

                hsl = slice(base, base + NVH[h])
                nc.scalar.activation(rowtmp[0:1, hsl], ssqD_row[0:1, hsl], AF.Sqrt)
                # ~51-ULP reciprocal (fp32-only op), then a cast to bf16 for
                # the K=1 broadcast matmul (plenty next to bf16 sim rounding)
                nc.vector.reciprocal_approx_fast(
                    invnD32[0:1, hsl], rowtmp[0:1, hsl]
                )
                nc.scalar.copy(invnD_row[0:1, hsl], invnD32[0:1, hsl])

                # broadcast 1/|D| across partitions and scale D straight out
                # of the projection PSUM (psd stays live until here)
                for (off, ln) in h_chunks:
                    gsl = slice(base + off, base + off + ln)
                    psb = psB.tile([128, 512], f32, name="psb", tag="psb")
                    nc.tensor.matmul(
                        psb[:, :ln], onesrow[:], invnD_row[:, gsl],
                        start=True, stop=True,
                    )
                    bc = dt_pool.tile([128, 512], bf16, name="bcast_sb",
                                      tag="bc", bufs=2)
                    nc.scalar.copy(bc[:, :ln], psb[:, :ln])
                    nc.vector.tensor_tensor(
                        DTn[:, gsl], psd[:, off:off + ln], bc[:, :ln],
                        op=ALU.mult,
                    )

        # ---------- phase Q+S: project query chunks, sim tiles interleaved --
        # Q-projection chunk j feeds sim tiles t=4j..4j+3; chunks are traced
        # two groups ahead of their sim tiles so the PE never starves the DVE
        # reduce pipeline.  The psQS pool's two [128, NVH] fp32 tensors are
        # shared between q-projection chunks (first 512 cols) and sim halves.
        with (
            tc.tile_pool(name="psQS", bufs=2, space="PSUM") as psQS,
            tc.tile_pool(name="ssQ", bufs=1, space="PSUM") as ssQ,
            tc.tile_pool(name="sqQ_pool", bufs=2) as sqQ_pool,
            tc.tile_pool(name="psO", bufs=1, space="PSUM") as psO,
            tc.tile_pool(name="m_pool", bufs=2) as m_pool,
        ):
            ssq = ssQ.tile([128, NTT], f32, name="ssq")
            psout = psO.tile([BPT, NTT * DPC], f32, name="psout")

            psq_live = {}

            def project_mm(j, ks):
                jg, r = divmod(j * 512, QG)
                if j not in psq_live:
                    psq_live[j] = psQS.tile([128, NVH[0]], f32, name="psq", tag="big")
                psq = psq_live[j]
                for k in ks:
                    nc.tensor.matmul(
                        psq[:, 0:512],
                        wt[:, k, :],
                        qts[(k, jg)][:, r:r + 512],
                        start=(k == 0),
                        stop=(k == KC - 1),
                    )

            def project(j):
                sl = slice(j * 512, (j + 1) * 512)
                psq = psq_live.pop(j)
                nc.scalar.copy(QT[:, sl], psq[:, 0:512])
                sq = sqQ_pool.tile([128, 512], bf16, name="sqq", tag="sqq")
                nc.scalar.activation(sq[:], psq[:, 0:512], AF.Square)
                for s in range(4):
                    col = j * 4 + s
                    nc.tensor.matmul(
                        ssq[:, col:col + 1],
                        sq[:, s * 128:(s + 1) * 128],
                        onescol[:],
                        start=True,
                        stop=True,
                    )
                # per-chunk 1/|Q| and the weighted block-ones lhsT
                csl = slice(j * 4, (j + 1) * 4)
                nc.scalar.activation(normQ[:, csl], ssq[:, csl], AF.Sqrt)
                nc.vector.reciprocal(invnQ[:, csl], normQ[:, csl])
                nc.vector.tensor_tensor(
                    lhsQ[:, csl, :],
                    qso[:].unsqueeze(1).broadcast_to((128, 4, BPT)),
                    invnQ[:, csl].unsqueeze(2).broadcast_to((128, 4, BPT)),
                    op=ALU.mult,
                )

            def simtile(t, weave=None):
                lq = QT[:, t * 128:(t + 1) * 128]
                mall = m_pool.tile([128, DPC], f32, name="mall", tag="mall")
                for h in range(2):
                    base = HB[h]
                    ps = psQS.tile([128, NVH[0]], f32, name="pssim", tag="big")
                    for (off, ln) in _chunks(NVH[h], 512):
                        nc.tensor.matmul(
                            ps[:, off:off + ln],
                            lq,
                            DTn[:, base + off:base + off + ln],
                            start=True,
                            stop=True,
                        )
                    nc.vector.reduce_max(
                        mall[:, h * (DPC // 2):(h + 1) * (DPC // 2)],
                        ps[:, 0:NVH[h]].rearrange("p (g v) -> p g v", v=NVS[h]),
                        axis=AX.X,
                    )
                    if weave:
                        weave(h)
                nc.tensor.matmul(
                    psout[:, t * DPC:(t + 1) * DPC],
                    lhsQ[:, t, :],
                    mall[:],
                    start=True,
                    stop=True,
                )

            project_mm(0, range(KC))
            project(0)
            project_mm(1, range(KC))
            project(1)
            loaded = 1
            for j in range(NQCH):
                # weave next-next chunk's six projection matmuls one at a
                # time between sim halves so the DVE reduce never starves
                kstep = iter(range(KC))

                def weave(h, _j=j, _ks=kstep):
                    if _j + 2 < NQCH:
                        k = next(_ks, None)
                        if k is not None:
                            project_mm(_j + 2, [k])

                if j + 2 < NQCH and (j + 2) // 2 > loaded:
                    load_jg((j + 2) // 2)
                    loaded = (j + 2) // 2
                for ti, t in enumerate(range(j * 4, (j + 1) * 4)):
                    simtile(t, weave=weave)
                    if j + 2 < NQCH and ti == 2:
                        project(j + 2)
            nc.scalar.copy(outstage[:], psout[:])
            nc.sync.dma_start(
                out_d[:, :].rearrange("(t f) c -> f t c", f=BPT),
                outstage[:].rearrange("f (t c) -> f t c", c=DPC),
            )
        qs_stack.__exit__(None, None, None)

    nc.compile()
    return nc


def _host_prep(q_hidden, d_hidden, W, d_mask):
    import ml_dtypes

    q = np.ascontiguousarray(np.asarray(q_hidden, dtype=np.float32))
    d = np.ascontiguousarray(np.asarray(d_hidden, dtype=np.float32))
    w = np.ascontiguousarray(np.asarray(W, dtype=np.float32))
    mask = np.asarray(d_mask, dtype=bool)

    def _pad8(x):
        x = max(int(x), 16)
        return min(int(-(-x // 8) * 8), ((LD + 7) // 8) * 8)

    # two length classes: sort docs by valid-token count, the 64 longest go
    # to each core's half A (padded to the global max), the 64 shortest to
    # half B (padded to the 65th-longest count) — ~8% fewer sim/reduce
    # elements than uniform padding, same instruction count
    nv = mask.sum(axis=1)
    order = np.argsort(-nv, kind="stable")
    NA = B // 2
    NV_A = _pad8(nv[order[0]])
    NV_B = _pad8(nv[order[NA]])
    # core m scores docs docids[m] (8 A-class then 8 B-class), in order
    docids = [
        np.concatenate([order[m * 8:(m + 1) * 8],
                        order[NA + m * 8:NA + (m + 1) * 8]])
        for m in range(NCORES)
    ]

    def _gather(c, NV):
        # valid tokens first, padded with dups of the first valid token
        # (duplicates never change a max)
        v = np.flatnonzero(mask[c])
        row = np.full(NV, v[0], dtype=np.intp)
        row[:min(len(v), NV)] = v[:NV]
        return d[c, row, :]                         # [NV, HID]

    bf = ml_dtypes.bfloat16
    qT = np.ascontiguousarray(q.reshape(TQ, HID).T.astype(bf))   # [HID, TQ]
    # W.T rearranged so the [128, KC, DIM] SBUF tile is one contiguous DMA:
    # wTp[p, k, d] = W[d, k*128+p]
    wT = np.ascontiguousarray(
        w.T.reshape(KC, 128, DIM).transpose(1, 0, 2).astype(bf)
    )
    dT_cores = []
    for m in range(NCORES):
        blk = np.concatenate(
            [_gather(c, NV_A) for c in docids[m][:8]]
            + [_gather(c, NV_B) for c in docids[m][8:]]
        )                                           # [8*NV_A + 8*NV_B, HID]
        dT_cores.append(np.ascontiguousarray(blk.T.astype(bf)))

    qso = np.zeros((128, 128 // LQ), dtype=np.float32)
    for p in range(128):
        qso[p, p // LQ] = 1.0
    onescol = np.ones((128, 1), dtype=bf)
    onesrow = np.ones((1, 128), dtype=bf)
    return NV_A, NV_B, docids, qT, wT, dT_cores, qso, onescol, onesrow


def kernel(q_hidden, d_hidden, W, d_mask):
    from concourse.bass_utils import run_bass_kernel_spmd

    NV_A, NV_B, docids, qT, wT, dT_cores, qso, onescol, onesrow = _host_prep(
        q_hidden, d_hidden, W, d_mask
    )
    nc = _build_program(NV_A, NV_B)

    in_maps = [
        {
            "qT": qT,
            "dT": dT_cores[m],
            "wT": wT,
            "qso": qso,
            "onescol": onescol,
            "onesrow": onesrow,
        }
        for m in range(NCORES)
    ]
    res = run_bass_kernel_spmd(nc, in_maps, core_ids=list(range(NCORES)))
    out = np.empty((B, B), dtype=np.float32)
    for m in range(NCORES):
        out[:, docids[m]] = res.results[m]["out"]
    return np.ascontiguousarray(out)


# revision 12
# speedup vs baseline: 1.2223x; 1.2223x over previous
"""ColBERT intra-batch MaxSim scoring kernel for 8 Trainium2 NeuronCores.

Math (see reference):
  Q = l2norm(q_hidden @ W.T)                       [B, LQ, DIM]
  D = l2norm(d_hidden @ W.T); D masked             [B, LD, DIM]
  sim[b,c,q,k] = Q[b,q]·D[c,k]; masked k -> -inf
  out[b,c] = sum_q max_k sim

Sharding: docs (dim c) are sharded 16-per-core; q_hidden/W replicated.
Each core computes its [B, 16] slice of the score matrix.

Design notes (v2):
  * The DVE reduce_max over the sim matrix is the hard floor: every sim
    element must pass through the vector engine once at ~1 elem/cycle/lane
    (~81us/core for 4096 q-tokens x 2432 doc-token cols / 128 lanes); no
    other engine can reduce along the free axis from PSUM. The kernel is
    therefore organized so DVE runs reduces back-to-back and everything
    else hides under them.
  * ALL activations/weights move and multiply in bf16 (fp32 PE matmuls are
    ~3.5x slower and fp32 DMA is 2x the bytes); PSUM accumulation stays
    fp32. Host casts inputs to bf16 after transposing.
  * Host pre-transposes activations to [HID, tokens] so every matmul has
    its contraction dim on partitions. The doc mask is folded away on the
    host: valid tokens gathered front, tail padded with dup of the first
    valid token (dups never change a max) -> no masking on device.
  * Q is NOT normalized before the sim matmul: max_k is invariant under a
    positive per-query scale, so 1/|Q| is folded into the block-ones
    lhsT of the final query-sum matmul.
  * D IS normalized before the sim matmul (1/|d_k| does not commute with
    max_k): ones-matmul sumsq -> sqrt (ACT) + fast reciprocal (DVE) ->
    K=1 ones outer-product matmul broadcasts 1/|D| to 128 partitions ->
    DVE multiply straight out of the projection PSUM into bf16 SBUF.
  * DVE reads at most ONE PSUM operand per instruction (HW rule), and
    only ~1 elem/cycle in every mode, so no fold tricks help; the plain
    grouped reduce (one per sim half-tile) is optimal.
"""

import os

import numpy as np

B, LQ, LD, HID, DIM = 128, 32, 256, 768, 128
NCORES = 8
DPC = B // NCORES          # docs per core
TQ = B * LQ                # total query tokens
KC = HID // 128            # contraction chunks for the projection


def _chunks(total, step):
    """[(off, len)] cut at `step` boundaries — a matmul's PSUM output must
    stay inside a single 512-float bank, so chunks may never straddle one."""
    return [(o, min(step, total - o)) for o in range(0, total, step)]


def _build_program(NV_A, NV_B):
    import concourse.bass as bass  # noqa: F401
    import concourse.tile as tile
    from concourse import bacc, mybir

    f32 = mybir.dt.float32
    bf16 = mybir.dt.bfloat16
    AF = mybir.ActivationFunctionType
    AX = mybir.AxisListType
    ALU = mybir.AluOpType

    # two doc classes: the 8 longest docs (padded to NV_A) in half A, the 8
    # shortest (padded to NV_B <= NV_A) in half B — same instruction count,
    # ~8% fewer sim/reduce elements than uniform padding
    NVH = [(DPC // 2) * NV_A, (DPC // 2) * NV_B]
    HB = [0, NVH[0]]               # half base offsets
    NVS = [NV_A, NV_B]
    NVT = NVH[0] + NVH[1]          # compacted doc tokens per core
    NQCH = TQ // 512        # q-projection column chunks
    NTT = TQ // 128         # sim lhsT tiles (query-token tiles)
    BPT = 128 // LQ         # batch entries per query-token tile
    QG = 1024               # qt DMA column-group width

    nc = bacc.Bacc(
        "TRN2",
        target_bir_lowering=False,
        debug=False,
        num_devices=NCORES,
    )

    qT_d = nc.dram_tensor("qT", [HID, TQ], bf16, kind="ExternalInput")
    dT_d = nc.dram_tensor("dT", [HID, NVT], bf16, kind="ExternalInput")
    wT_d = nc.dram_tensor("wT", [128, KC, DIM], bf16, kind="ExternalInput")
    qso_d = nc.dram_tensor("qso", [128, BPT], f32, kind="ExternalInput")
    onescol_d = nc.dram_tensor("onescol", [128, 1], bf16, kind="ExternalInput")
    onesrow_d = nc.dram_tensor("onesrow", [1, 128], bf16, kind="ExternalInput")
    out_d = nc.dram_tensor("out", [B, DPC], f32, kind="ExternalOutput")

    with tile.TileContext(nc) as tc, tc.tile_pool(name="persist", bufs=1) as per:
        # --- constants + persistent SBUF tensors ---------------------------
        wt = per.tile([128, KC, DIM], bf16, name="wt")
        qso = per.tile([128, BPT], f32, name="qso")
        onescol = per.tile([128, 1], bf16, name="onescol")
        onesrow = per.tile([1, 128], bf16, name="onesrow")
        QT = per.tile([128, TQ], bf16, name="QT")         # q-proj [d, t] unnormalized
        DTn = per.tile([128, NVT], bf16, name="DTn")      # normalized d-proj
        invnQ = per.tile([128, NTT], f32, name="invnQ")   # 1/|Q| per query token
        normQ = per.tile([128, NTT], f32, name="normQ")
        lhsQ = per.tile([128, NTT, BPT], f32, name="lhsQ")  # blockones * 1/|Q|
        ssqD_row = per.tile([1, NVT], f32, name="ssqD_row")
        invnD32 = per.tile([1, NVT], f32, name="invnD32")
        invnD_row = per.tile([1, NVT], bf16, name="invnD_row")
        rowtmp = per.tile([1, NVT], f32, name="rowtmp")
        outstage = per.tile([BPT, NTT * DPC], f32, name="outstage")

        # constants go first on the gpsimd queue so wt is resident before
        # the first projection matmul; dT halves ride two queues in parallel
        # (sync: half A, gpsimd: half B) so phase D is DMA-gated for only
        # ~2MB per queue; qt jg0 rides the otherwise-idle scalar queue
        nc.gpsimd.dma_start(wt[:], wT_d[:, :, :])
        nc.gpsimd.dma_start(qso[:], qso_d[:, :])
        nc.gpsimd.dma_start(onescol[:], onescol_d[:, :])
        nc.gpsimd.dma_start(onesrow[:], onesrow_d[:, :])

        # ---------------- phase D: project + normalize doc tokens ---------
        # dT halves land on the sync queue (nothing else competes there);
        # qt column groups land on the gpsimd queue. Scalar/vector issue no
        # DMAs — their cycles belong to copies and reduces.
        qs_stack = tc.tile_pool(name="qt_pool", bufs=1)
        qt_pool = qs_stack.__enter__()
        qts = {}

        def load_jg(jg, eng=None):
            for k in range(KC):
                t_ = qt_pool.tile(
                    [128, QG], bf16, name=f"qt{k}_{jg}", tag=f"qt{k}",
                    bufs=2,
                )
                (eng or nc.gpsimd).dma_start(
                    t_[:], qT_d[k * 128:(k + 1) * 128, jg * QG:(jg + 1) * QG]
                )
                qts[(k, jg)] = t_

        # NOTE: scalar may only issue a FEW upfront DMAs — a back-pressured
        # issue blocks its in-order queue and stalls the phase-D squares
        # (measured +25us on the ramp with 12 queued issues).

        with (
            tc.tile_pool(name="dt_pool", bufs=1) as dt_pool,
            tc.tile_pool(name="psD", bufs=1, space="PSUM") as psD,
            tc.tile_pool(name="ssD", bufs=1, space="PSUM") as ssD,
            tc.tile_pool(name="sqD_pool", bufs=2) as sqD_pool,
            tc.tile_pool(name="psB", bufs=1, space="PSUM") as psB,
        ):
            # dT rides BOTH the sync and gpsimd queues, k-parity interleaved
            # with ALL of half A ahead of half B, so the first sim tiles fire
            # ~7us after half A lands; qt jg0+jg1 ride the scalar queue whose
            # 12 issues clear before the first phase-D square needs the ACT
            dts = {}
            for h in range(2):
                for k in range(KC):
                    dtk = dt_pool.tile(
                        [128, NVH[h]], bf16, name=f"dt{k}_{h}", tag=f"dt{k}_{h}"
                    )
                    eng = nc.sync if k % 2 == 0 else nc.gpsimd
                    eng.dma_start(
                        dtk[:],
                        dT_d[k * 128:(k + 1) * 128, HB[h]:HB[h] + NVH[h]],
                    )
                    dts[(k, h)] = dtk
            load_jg(0, eng=nc.scalar)
            load_jg(1, eng=nc.scalar)

            for h in range(2):
                base = HB[h]
                h_chunks = _chunks(NVH[h], 512)
                psd = psD.tile([128, NVH[h]], f32, name="psd", tag=f"psd{h}")
                for k in range(KC):
                    for (off, ln) in h_chunks:
                        nc.tensor.matmul(
                            psd[:, off:off + ln],
                            wt[:, k, :],
                            dts[(k, h)][:, off:off + ln],
                            start=(k == 0),
                            stop=(k == KC - 1),
                        )
                # per-chunk norm pipeline: the whole sumsq->sqrt->recip->
                # broadcast->scale chain runs chunk-local so DTn columns are
                # ready ~3us after their psd chunk instead of ~8
                for (off, ln) in h_chunks:
                    sl = slice(off, off + ln)
                    gsl = slice(base + off, base + off + ln)
                    sq = sqD_pool.tile([128, 512], bf16, name="sqd", tag="sq")
                    nc.scalar.activation(sq[:, :ln], psd[:, sl], AF.Square)
                    ssd = ssD.tile([1, 512], f32, name="ssd", tag="ssd")
                    nc.tensor.matmul(
                        ssd[:, :ln], onescol[:], sq[:, :ln], start=True, stop=True
                    )
                    nc.scalar.activation(rowtmp[0:1, gsl], ssd[0:1, :ln], AF.Sqrt)
                    # ~51-ULP reciprocal (fp32-only op) + cast to bf16 for the
                    # K=1 broadcast matmul (plenty next to bf16 sim rounding)
                    nc.vector.reciprocal_approx_fast(
                        invnD32[0:1, gsl], rowtmp[0:1, gsl]
                    )
                    nc.scalar.copy(invnD_row[0:1, gsl], invnD32[0:1, gsl])
                    psb = psB.tile([128, 512], f32, name="psb", tag="psb")
                    nc.tensor.matmul(
                        psb[:, :ln], onesrow[:], invnD_row[:, gsl],
                        start=True, stop=True,
                    )
                    bc = dt_pool.tile([128, 512], bf16, name="bcast_sb",
                                      tag="bc", bufs=2)
                    nc.scalar.copy(bc[:, :ln], psb[:, :ln])
                    nc.vector.tensor_tensor(
                        DTn[:, gsl], psd[:, sl], bc[:, :ln],
                        op=ALU.mult,
                    )

        # ---------- phase Q+S: project query chunks, sim tiles interleaved --
        # Q-projection chunk j covers sim tiles 4j..4j+3. Chunks are burst
        # two ahead of their tiles; the 512-col projection PSUM has its own
        # single bank (a burst holds it ~1 tile) so the two big sim tensors
        # ping-pong PE writes against DVE reduces without a third claimant.
        # PSUM budget: 2x3 (sim) + 1 (qproj) + 1 (ssq + psout windows) = 8.
        with (
            tc.tile_pool(name="psQS", bufs=2, space="PSUM") as psQS,
            tc.tile_pool(name="psQ", bufs=1, space="PSUM") as psQ,
            tc.tile_pool(name="psM", bufs=1, space="PSUM") as psM,
            tc.tile_pool(name="sqQ_pool", bufs=2) as sqQ_pool,
            tc.tile_pool(name="m_pool", bufs=2) as m_pool,
        ):
            # one bank holds the Q sumsq columns (cols 0:NTT) and two
            # rotating [BPT, DPC] psout windows (cols NTT:NTT+2*DPC)
            misc = psM.tile([128, 512], f32, name="misc")

            psq_live = {}
            mall_live = {}

            def project_mm(j):
                jg, r = divmod(j * 512, QG)
                psq = psQ.tile([128, 512], f32, name="psq", tag="psq")
                psq_live[j] = psq
                for k in range(KC):
                    nc.tensor.matmul(
                        psq[:, 0:512],
                        wt[:, k, :],
                        qts[(k, jg)][:, r:r + 512],
                        start=(k == 0),
                        stop=(k == KC - 1),
                    )

            def project(j):
                sl = slice(j * 512, (j + 1) * 512)
                psq = psq_live.pop(j)
                nc.scalar.copy(QT[:, sl], psq[:, 0:512])
                sq = sqQ_pool.tile([128, 512], bf16, name="sqq", tag="sqq")
                nc.scalar.activation(sq[:], psq[:, 0:512], AF.Square)
                for sx in range(4):
                    col = j * 4 + sx
                    nc.tensor.matmul(
                        misc[:, col:col + 1],
                        sq[:, sx * 128:(sx + 1) * 128],
                        onescol[:],
                        start=True,
                        stop=True,
                    )
                # per-chunk 1/|Q| and the weighted block-ones lhsT
                csl = slice(j * 4, (j + 1) * 4)
                nc.scalar.activation(normQ[:, csl], misc[:, csl], AF.Sqrt)
                nc.vector.reciprocal(invnQ[:, csl], normQ[:, csl])
                nc.vector.tensor_tensor(
                    lhsQ[:, csl, :],
                    qso[:].unsqueeze(1).broadcast_to((128, 4, BPT)),
                    invnQ[:, csl].unsqueeze(2).broadcast_to((128, 4, BPT)),
                    op=ALU.mult,
                )

            def simhalf(t, h):
                lq = QT[:, t * 128:(t + 1) * 128]
                if h == 0:
                    mall_live[t] = m_pool.tile([128, DPC], f32, name="mall",
                                               tag="mall")
                mall = mall_live[t]
                base = HB[h]
                ps = psQS.tile([128, NVH[0]], f32, name="pssim", tag="big")
                for (off, ln) in _chunks(NVH[h], 512):
                    nc.tensor.matmul(
                        ps[:, off:off + ln],
                        lq,
                        DTn[:, base + off:base + off + ln],
                        start=True,
                        stop=True,
                    )
                nc.vector.reduce_max(
                    mall[:, h * (DPC // 2):(h + 1) * (DPC // 2)],
                    ps[:, 0:NVH[h]].rearrange("p (g v) -> p g v", v=NVS[h]),
                    axis=AX.X,
                )

            def finish_tile(t):
                mall = mall_live.pop(t)
                w = NTT + (t % 2) * DPC
                nc.tensor.matmul(
                    misc[0:BPT, w:w + DPC],
                    lhsQ[:, t, :],
                    mall[:],
                    start=True,
                    stop=True,
                )
                nc.scalar.copy(
                    outstage[:, t * DPC:(t + 1) * DPC], misc[0:BPT, w:w + DPC]
                )

            # tiles 0-3 ride chunk 0; both A-halves run before the B-halves
            # so the reduce stream starts as soon as half A's DTn lands
            project_mm(0)
            project(0)
            simhalf(0, 0)
            simhalf(1, 0)
            simhalf(0, 1)
            simhalf(1, 1)
            project_mm(1)
            finish_tile(0)
            finish_tile(1)
            project(1)
            simhalf(2, 0)
            simhalf(2, 1)
            project_mm(2)
            finish_tile(2)
            simhalf(3, 0)
            simhalf(3, 1)
            finish_tile(3)
            project(2)
            load_jg(2)
            loaded = 2
            for j in range(1, NQCH):
                if j + 3 < NQCH // 2 * 2 and (j + 3) // 2 > loaded:
                    load_jg((j + 3) // 2)
                    loaded = (j + 3) // 2
                for ti, t in enumerate(range(j * 4, (j + 1) * 4)):
                    simhalf(t, 0)
                    if ti == 1 and j + 2 < NQCH:
                        project_mm(j + 2)
                    simhalf(t, 1)
                    finish_tile(t)
                    if ti == 2 and j + 2 < NQCH:
                        project(j + 2)
            nc.sync.dma_start(
                out_d[:, :].rearrange("(t f) c -> f t c", f=BPT),
                outstage[:].rearrange("f (t c) -> f t c", c=DPC),
            )
        qs_stack.__exit__(None, None, None)

    nc.compile()
    return nc


def _host_prep(q_hidden, d_hidden, W, d_mask):
    import ml_dtypes

    q = np.ascontiguousarray(np.asarray(q_hidden, dtype=np.float32))
    d = np.ascontiguousarray(np.asarray(d_hidden, dtype=np.float32))
    w = np.ascontiguousarray(np.asarray(W, dtype=np.float32))
    mask = np.asarray(d_mask, dtype=bool)

    def _pad8(x):
        x = max(int(x), 16)
        return min(int(-(-x // 8) * 8), ((LD + 7) // 8) * 8)

    # two length classes: sort docs by valid-token count, the 64 longest go
    # to each core's half A (padded to the global max), the 64 shortest to
    # half B (padded to the 65th-longest count) — ~8% fewer sim/reduce
    # elements than uniform padding, same instruction count
    nv = mask.sum(axis=1)
    order = np.argsort(-nv, kind="stable")
    NA = B // 2
    NV_A = _pad8(nv[order[0]])
    NV_B = _pad8(nv[order[NA]])
    # core m scores docs docids[m] (8 A-class then 8 B-class), in order
    docids = [
        np.concatenate([order[m * 8:(m + 1) * 8],
                        order[NA + m * 8:NA + (m + 1) * 8]])
        for m in range(NCORES)
    ]

    def _gather(c, NV):
        # valid tokens first, padded with dups of the first valid token
        # (duplicates never change a max)
        v = np.flatnonzero(mask[c])
        row = np.full(NV, v[0], dtype=np.intp)
        row[:min(len(v), NV)] = v[:NV]
        return d[c, row, :]                         # [NV, HID]

    bf = ml_dtypes.bfloat16
    qT = np.ascontiguousarray(q.reshape(TQ, HID).T.astype(bf))   # [HID, TQ]
    # W.T rearranged so the [128, KC, DIM] SBUF tile is one contiguous DMA:
    # wTp[p, k, d] = W[d, k*128+p]
    wT = np.ascontiguousarray(
        w.T.reshape(KC, 128, DIM).transpose(1, 0, 2).astype(bf)
    )
    dT_cores = []
    for m in range(NCORES):
        blk = np.concatenate(
            [_gather(c, NV_A) for c in docids[m][:8]]
            + [_gather(c, NV_B) for c in docids[m][8:]]
        )                                           # [8*NV_A + 8*NV_B, HID]
        dT_cores.append(np.ascontiguousarray(blk.T.astype(bf)))

    qso = np.zeros((128, 128 // LQ), dtype=np.float32)
    for p in range(128):
        qso[p, p // LQ] = 1.0
    onescol = np.ones((128, 1), dtype=bf)
    onesrow = np.ones((1, 128), dtype=bf)
    return NV_A, NV_B, docids, qT, wT, dT_cores, qso, onescol, onesrow


def kernel(q_hidden, d_hidden, W, d_mask):
    from concourse.bass_utils import run_bass_kernel_spmd

    NV_A, NV_B, docids, qT, wT, dT_cores, qso, onescol, onesrow = _host_prep(
        q_hidden, d_hidden, W, d_mask
    )
    nc = _build_program(NV_A, NV_B)

    in_maps = [
        {
            "qT": qT,
            "dT": dT_cores[m],
            "wT": wT,
            "qso": qso,
            "onescol": onescol,
            "onesrow": onesrow,
        }
        for m in range(NCORES)
    ]
    res = run_bass_kernel_spmd(nc, in_maps, core_ids=list(range(NCORES)))
    out = np.empty((B, B), dtype=np.float32)
    for m in range(NCORES):
        out[:, docids[m]] = res.results[m]["out"]
    return np.ascontiguousarray(out)


# revision 14
# speedup vs baseline: 1.2858x; 1.0519x over previous
"""ColBERT intra-batch MaxSim scoring kernel for 8 Trainium2 NeuronCores.

Math (see reference):
  Q = l2norm(q_hidden @ W.T)                       [B, LQ, DIM]
  D = l2norm(d_hidden @ W.T); D masked             [B, LD, DIM]
  sim[b,c,q,k] = Q[b,q]·D[c,k]; masked k -> -inf
  out[b,c] = sum_q max_k sim

Sharding: docs (dim c) are sharded 16-per-core; q_hidden/W replicated.
Each core computes its [B, 16] slice of the score matrix.

Design notes (v2):
  * The DVE reduce_max over the sim matrix is the hard floor: every sim
    element must pass through the vector engine once at ~1 elem/cycle/lane
    (~81us/core for 4096 q-tokens x 2432 doc-token cols / 128 lanes); no
    other engine can reduce along the free axis from PSUM. The kernel is
    therefore organized so DVE runs reduces back-to-back and everything
    else hides under them.
  * ALL activations/weights move and multiply in bf16 (fp32 PE matmuls are
    ~3.5x slower and fp32 DMA is 2x the bytes); PSUM accumulation stays
    fp32. Host casts inputs to bf16 after transposing.
  * Host pre-transposes activations to [HID, tokens] so every matmul has
    its contraction dim on partitions. The doc mask is folded away on the
    host: valid tokens gathered front, tail padded with dup of the first
    valid token (dups never change a max) -> no masking on device.
  * Q is NOT normalized before the sim matmul: max_k is invariant under a
    positive per-query scale, so 1/|Q| is folded into the block-ones
    lhsT of the final query-sum matmul.
  * D IS normalized before the sim matmul (1/|d_k| does not commute with
    max_k): ones-matmul sumsq -> sqrt (ACT) + fast reciprocal (DVE) ->
    K=1 ones outer-product matmul broadcasts 1/|D| to 128 partitions ->
    DVE multiply straight out of the projection PSUM into bf16 SBUF.
  * DVE reads at most ONE PSUM operand per instruction (HW rule), and
    only ~1 elem/cycle in every mode, so no fold tricks help; the plain
    grouped reduce (one per sim half-tile) is optimal.
"""

import os

import numpy as np

B, LQ, LD, HID, DIM = 128, 32, 256, 768, 128
NCORES = 8
DPC = B // NCORES          # docs per core
TQ = B * LQ                # total query tokens
KC = HID // 128            # contraction chunks for the projection


def _chunks(total, step):
    """[(off, len)] cut at `step` boundaries — a matmul's PSUM output must
    stay inside a single 512-float bank, so chunks may never straddle one."""
    return [(o, min(step, total - o)) for o in range(0, total, step)]


def _build_program(NV_A, NV_B):
    import concourse.bass as bass  # noqa: F401
    import concourse.tile as tile
    from concourse import bacc, mybir

    f32 = mybir.dt.float32
    bf16 = mybir.dt.bfloat16
    AF = mybir.ActivationFunctionType
    AX = mybir.AxisListType
    ALU = mybir.AluOpType

    # two doc classes: the 8 longest docs (padded to NV_A) in half A, the 8
    # shortest (padded to NV_B <= NV_A) in half B — same instruction count,
    # ~8% fewer sim/reduce elements than uniform padding
    NVH = [(DPC // 2) * NV_A, (DPC // 2) * NV_B]
    HB = [0, NVH[0]]               # half base offsets
    NVS = [NV_A, NV_B]
    NVT = NVH[0] + NVH[1]          # compacted doc tokens per core
    NQCH = TQ // 512        # q-projection column chunks
    NTT = TQ // 128         # sim lhsT tiles (query-token tiles)
    BPT = 128 // LQ         # batch entries per query-token tile
    QG = 1024               # qt DMA column-group width

    nc = bacc.Bacc(
        "TRN2",
        target_bir_lowering=False,
        debug=False,
        num_devices=NCORES,
    )

    qT_d = nc.dram_tensor("qT", [HID, TQ], bf16, kind="ExternalInput")
    dT_d = nc.dram_tensor("dT", [HID, NVT], bf16, kind="ExternalInput")
    wT_d = nc.dram_tensor("wT", [128, KC, DIM], bf16, kind="ExternalInput")
    qso_d = nc.dram_tensor("qso", [128, BPT], f32, kind="ExternalInput")
    onescol_d = nc.dram_tensor("onescol", [128, 1], bf16, kind="ExternalInput")
    onesrow_d = nc.dram_tensor("onesrow", [1, 128], bf16, kind="ExternalInput")
    out_d = nc.dram_tensor("out", [B, DPC], f32, kind="ExternalOutput")

    with tile.TileContext(nc) as tc, tc.tile_pool(name="persist", bufs=1) as per:
        # --- constants + persistent SBUF tensors ---------------------------
        wt = per.tile([128, KC, DIM], bf16, name="wt")
        qso = per.tile([128, BPT], f32, name="qso")
        onescol = per.tile([128, 1], bf16, name="onescol")
        onesrow = per.tile([1, 128], bf16, name="onesrow")
        QT = per.tile([128, TQ], bf16, name="QT")         # q-proj [d, t] unnormalized
        DTn = per.tile([128, NVT], bf16, name="DTn")      # normalized d-proj
        invnQ = per.tile([128, NTT], f32, name="invnQ")   # 1/|Q| per query token
        normQ = per.tile([128, NTT], f32, name="normQ")
        lhsQ = per.tile([128, NTT, BPT], f32, name="lhsQ")  # blockones * 1/|Q|
        ssqD_row = per.tile([1, NVT], f32, name="ssqD_row")
        invnD32 = per.tile([1, NVT], f32, name="invnD32")
        invnD_row = per.tile([1, NVT], bf16, name="invnD_row")
        rowtmp = per.tile([1, NVT], f32, name="rowtmp")
        outstage = per.tile([BPT, NTT * DPC], f32, name="outstage")

        # constants go first on the gpsimd queue so wt is resident before
        # the first projection matmul; dT halves ride two queues in parallel
        # (sync: half A, gpsimd: half B) so phase D is DMA-gated for only
        # ~2MB per queue; qt jg0 rides the otherwise-idle scalar queue
        nc.gpsimd.dma_start(wt[:], wT_d[:, :, :])
        nc.gpsimd.dma_start(qso[:], qso_d[:, :])
        nc.gpsimd.dma_start(onescol[:], onescol_d[:, :])
        nc.gpsimd.dma_start(onesrow[:], onesrow_d[:, :])

        # ---------------- phase D: project + normalize doc tokens ---------
        # dT halves land on the sync queue (nothing else competes there);
        # qt column groups land on the gpsimd queue. Scalar/vector issue no
        # DMAs — their cycles belong to copies and reduces.
        qs_stack = tc.tile_pool(name="qt_pool", bufs=1)
        qt_pool = qs_stack.__enter__()
        qts = {}

        def load_jg(jg, eng=None):
            for k in range(KC):
                t_ = qt_pool.tile(
                    [128, QG], bf16, name=f"qt{k}_{jg}", tag=f"qt{k}",
                    bufs=2,
                )
                (eng or nc.gpsimd).dma_start(
                    t_[:], qT_d[k * 128:(k + 1) * 128, jg * QG:(jg + 1) * QG]
                )
                qts[(k, jg)] = t_

        # NOTE: scalar may only issue a FEW upfront DMAs — a back-pressured
        # issue blocks its in-order queue and stalls the phase-D squares
        # (measured +25us on the ramp with 12 queued issues).

        with (
            tc.tile_pool(name="dt_pool", bufs=1) as dt_pool,
            tc.tile_pool(name="psD", bufs=1, space="PSUM") as psD,
            tc.tile_pool(name="ssD", bufs=1, space="PSUM") as ssD,
            tc.tile_pool(name="sqD_pool", bufs=2) as sqD_pool,
            tc.tile_pool(name="psB", bufs=1, space="PSUM") as psB,
        ):
            # dT half A rides ALL THREE queues (sync: k0/k2/k4, gpsimd:
            # k1/k3, scalar: k5 first) so its projection input lands ~8.5us;
            # half B follows on sync+gpsimd; jg0 rides scalar behind A-k5 and
            # jg1 rides sync behind half B. Scalar gets only 7 issues — more
            # blocks its in-order queue and stalls the phase-D squares.
            dts = {}
            for h in range(2):
                for k in range(KC):
                    dts[(k, h)] = dt_pool.tile(
                        [128, NVH[h]], bf16, name=f"dt{k}_{h}", tag=f"dt{k}_{h}"
                    )

            def load_dt(h, ks, eng):
                for k in ks:
                    eng.dma_start(
                        dts[(k, h)][:],
                        dT_d[k * 128:(k + 1) * 128, HB[h]:HB[h] + NVH[h]],
                    )

            load_dt(0, [5], nc.scalar)
            load_dt(0, [0, 2, 4], nc.sync)
            load_dt(0, [1, 3], nc.gpsimd)
            load_jg(0, eng=nc.scalar)
            load_dt(1, [0, 2, 4], nc.sync)
            load_dt(1, [1, 3, 5], nc.gpsimd)
            load_jg(1, eng=nc.sync)

            for h in range(2):
                base = HB[h]
                h_chunks = _chunks(NVH[h], 512)
                psd = psD.tile([128, NVH[h]], f32, name="psd", tag=f"psd{h}")
                for k in range(KC):
                    for (off, ln) in h_chunks:
                        nc.tensor.matmul(
                            psd[:, off:off + ln],
                            wt[:, k, :],
                            dts[(k, h)][:, off:off + ln],
                            start=(k == 0),
                            stop=(k == KC - 1),
                        )
                # half-wide extract: ACT has ~530ns fixed cost per
                # instruction, so one full-width op per step beats a
                # chunk-local pipeline by ~5us of pure overhead
                hsl = slice(base, base + NVH[h])
                sq = sqD_pool.tile([128, NVH[0]], bf16, name="sqd", tag="sq")
                nc.scalar.activation(sq[:, 0:NVH[h]], psd[:], AF.Square)
                for (off, ln) in h_chunks:
                    ssd = ssD.tile([1, 512], f32, name="ssd", tag="ssd")
                    nc.tensor.matmul(
                        ssd[:, :ln], onescol[:], sq[:, off:off + ln],
                        start=True, stop=True,
                    )
                    nc.scalar.copy(ssqD_row[:, base + off:base + off + ln],
                                   ssd[:, :ln])
                nc.scalar.activation(rowtmp[0:1, hsl], ssqD_row[0:1, hsl],
                                     AF.Sqrt)
                # ~51-ULP reciprocal (fp32-only op) + cast to bf16 for the
                # K=1 broadcast matmul (plenty next to bf16 sim rounding)
                nc.vector.reciprocal_approx_fast(
                    invnD32[0:1, hsl], rowtmp[0:1, hsl]
                )
                nc.scalar.copy(invnD_row[0:1, hsl], invnD32[0:1, hsl])
                bc = dt_pool.tile([128, NVH[0]], bf16, name="bcast_sb",
                                  tag="bc")
                for (off, ln) in h_chunks:
                    psb = psB.tile([128, 512], f32, name="psb", tag="psb")
                    nc.tensor.matmul(
                        psb[:, :ln], onesrow[:],
                        invnD_row[:, base + off:base + off + ln],
                        start=True, stop=True,
                    )
                    nc.scalar.copy(bc[:, off:off + ln], psb[:, :ln])
                nc.vector.tensor_tensor(
                    DTn[:, hsl], psd[:], bc[:, 0:NVH[h]], op=ALU.mult
                )

        # ---------- phase Q+S: project query chunks, sim tiles interleaved --
        # Q-projection chunk j covers sim tiles 4j..4j+3. Chunks are burst
        # two ahead of their tiles; the 512-col projection PSUM has its own
        # single bank (a burst holds it ~1 tile) so the two big sim tensors
        # ping-pong PE writes against DVE reduces without a third claimant.
        # PSUM budget: 2x3 (sim) + 1 (qproj) + 1 (ssq + psout windows) = 8.
        with (
            tc.tile_pool(name="psQS", bufs=2, space="PSUM") as psQS,
            tc.tile_pool(name="psQ", bufs=1, space="PSUM") as psQ,
            tc.tile_pool(name="psM", bufs=1, space="PSUM") as psM,
            tc.tile_pool(name="sqQ_pool", bufs=2) as sqQ_pool,
            tc.tile_pool(name="m_pool", bufs=4) as m_pool,
        ):
            # one bank holds the Q sumsq columns (cols 0:NTT) and two
            # rotating [BPT, DPC] psout windows (cols NTT:NTT+2*DPC)
            misc = psM.tile([128, 512], f32, name="misc")

            psq_live = {}
            mall_live = {}

            def project_mm(j):
                jg, r = divmod(j * 512, QG)
                psq = psQ.tile([128, 512], f32, name="psq", tag="psq")
                psq_live[j] = psq
                for k in range(KC):
                    nc.tensor.matmul(
                        psq[:, 0:512],
                        wt[:, k, :],
                        qts[(k, jg)][:, r:r + 512],
                        start=(k == 0),
                        stop=(k == KC - 1),
                    )

            def project(j):
                sl = slice(j * 512, (j + 1) * 512)
                psq = psq_live.pop(j)
                nc.scalar.copy(QT[:, sl], psq[:, 0:512])
                sq = sqQ_pool.tile([128, 512], bf16, name="sqq", tag="sqq")
                nc.scalar.activation(sq[:], psq[:, 0:512], AF.Square)
                for sx in range(4):
                    col = j * 4 + sx
                    nc.tensor.matmul(
                        misc[:, col:col + 1],
                        sq[:, sx * 128:(sx + 1) * 128],
                        onescol[:],
                        start=True,
                        stop=True,
                    )
                # per-chunk 1/|Q| and the weighted block-ones lhsT
                csl = slice(j * 4, (j + 1) * 4)
                nc.scalar.activation(normQ[:, csl], misc[:, csl], AF.Sqrt)
                nc.vector.reciprocal(invnQ[:, csl], normQ[:, csl])
                nc.vector.tensor_tensor(
                    lhsQ[:, csl, :],
                    qso[:].unsqueeze(1).broadcast_to((128, 4, BPT)),
                    invnQ[:, csl].unsqueeze(2).broadcast_to((128, 4, BPT)),
                    op=ALU.mult,
                )

            def simhalf(t, h):
                lq = QT[:, t * 128:(t + 1) * 128]
                if h == 0:
                    mall_live[t] = m_pool.tile([128, DPC], f32, name="mall",
                                               tag="mall")
                mall = mall_live[t]
                base = HB[h]
                ps = psQS.tile([128, NVH[0]], f32, name="pssim", tag="big")
                for (off, ln) in _chunks(NVH[h], 512):
                    nc.tensor.matmul(
                        ps[:, off:off + ln],
                        lq,
                        DTn[:, base + off:base + off + ln],
                        start=True,
                        stop=True,
                    )
                nc.vector.reduce_max(
                    mall[:, h * (DPC // 2):(h + 1) * (DPC // 2)],
                    ps[:, 0:NVH[h]].rearrange("p (g v) -> p g v", v=NVS[h]),
                    axis=AX.X,
                )

            def finish_tile(t):
                mall = mall_live.pop(t)
                w = NTT + (t % 2) * DPC
                nc.tensor.matmul(
                    misc[0:BPT, w:w + DPC],
                    lhsQ[:, t, :],
                    mall[:],
                    start=True,
                    stop=True,
                )
                nc.scalar.copy(
                    outstage[:, t * DPC:(t + 1) * DPC], misc[0:BPT, w:w + DPC]
                )

            # tiles 0-3 ride chunk 0; ALL FOUR A-halves run before any
            # B-half — half B's DTn lands ~5us after half A's, and four A
            # reduces (~5.6us) cover exactly that window
            project_mm(0)
            project(0)
            simhalf(0, 0)
            simhalf(1, 0)
            project_mm(1)
            simhalf(2, 0)
            simhalf(3, 0)
            project(1)
            simhalf(0, 1)
            simhalf(1, 1)
            project_mm(2)
            simhalf(2, 1)
            simhalf(3, 1)
            finish_tile(0)
            finish_tile(1)
            finish_tile(2)
            finish_tile(3)
            project(2)
            load_jg(2)
            loaded = 2
            for j in range(1, NQCH):
                if j + 3 < NQCH // 2 * 2 and (j + 3) // 2 > loaded:
                    load_jg((j + 3) // 2)
                    loaded = (j + 3) // 2
                for ti, t in enumerate(range(j * 4, (j + 1) * 4)):
                    simhalf(t, 0)
                    if ti == 1 and j + 2 < NQCH:
                        project_mm(j + 2)
                    simhalf(t, 1)
                    finish_tile(t)
                    if ti == 2 and j + 2 < NQCH:
                        project(j + 2)
            nc.sync.dma_start(
                out_d[:, :].rearrange("(t f) c -> f t c", f=BPT),
                outstage[:].rearrange("f (t c) -> f t c", c=DPC),
            )
        qs_stack.__exit__(None, None, None)

    nc.compile()
    return nc


def _host_prep(q_hidden, d_hidden, W, d_mask):
    import ml_dtypes

    q = np.ascontiguousarray(np.asarray(q_hidden, dtype=np.float32))
    d = np.ascontiguousarray(np.asarray(d_hidden, dtype=np.float32))
    w = np.ascontiguousarray(np.asarray(W, dtype=np.float32))
    mask = np.asarray(d_mask, dtype=bool)

    def _pad8(x):
        x = max(int(x), 16)
        return min(int(-(-x // 8) * 8), ((LD + 7) // 8) * 8)

    # two length classes: sort docs by valid-token count, the 64 longest go
    # to each core's half A (padded to the global max), the 64 shortest to
    # half B (padded to the 65th-longest count) — ~8% fewer sim/reduce
    # elements than uniform padding, same instruction count
    nv = mask.sum(axis=1)
    order = np.argsort(-nv, kind="stable")
    NA = B // 2
    NV_A = _pad8(nv[order[0]])
    NV_B = _pad8(nv[order[NA]])
    # core m scores docs docids[m] (8 A-class then 8 B-class), in order
    docids = [
        np.concatenate([order[m * 8:(m + 1) * 8],
                        order[NA + m * 8:NA + (m + 1) * 8]])
        for m in range(NCORES)
    ]

    def _gather(c, NV):
        # valid tokens first, padded with dups of the first valid token
        # (duplicates never change a max)
        v = np.flatnonzero(mask[c])
        row = np.full(NV, v[0], dtype=np.intp)
        row[:min(len(v), NV)] = v[:NV]
        return d[c, row, :]                         # [NV, HID]

    bf = ml_dtypes.bfloat16
    qT = np.ascontiguousarray(q.reshape(TQ, HID).T.astype(bf))   # [HID, TQ]
    # W.T rearranged so the [128, KC, DIM] SBUF tile is one contiguous DMA:
    # wTp[p, k, d] = W[d, k*128+p]
    wT = np.ascontiguousarray(
        w.T.reshape(KC, 128, DIM).transpose(1, 0, 2).astype(bf)
    )
    dT_cores = []
    for m in range(NCORES):
        blk = np.concatenate(
            [_gather(c, NV_A) for c in docids[m][:8]]
            + [_gather(c, NV_B) for c in docids[m][8:]]
        )                                           # [8*NV_A + 8*NV_B, HID]
        dT_cores.append(np.ascontiguousarray(blk.T.astype(bf)))

    qso = np.zeros((128, 128 // LQ), dtype=np.float32)
    for p in range(128):
        qso[p, p // LQ] = 1.0
    onescol = np.ones((128, 1), dtype=bf)
    onesrow = np.ones((1, 128), dtype=bf)
    return NV_A, NV_B, docids, qT, wT, dT_cores, qso, onescol, onesrow


def kernel(q_hidden, d_hidden, W, d_mask):
    from concourse.bass_utils import run_bass_kernel_spmd

    NV_A, NV_B, docids, qT, wT, dT_cores, qso, onescol, onesrow = _host_prep(
        q_hidden, d_hidden, W, d_mask
    )
    nc = _build_program(NV_A, NV_B)

    in_maps = [
        {
            "qT": qT,
            "dT": dT_cores[m],
            "wT": wT,
            "qso": qso,
            "onescol": onescol,
            "onesrow": onesrow,
        }
        for m in range(NCORES)
    ]
    res = run_bass_kernel_spmd(nc, in_maps, core_ids=list(range(NCORES)))
    out = np.empty((B, B), dtype=np.float32)
    for m in range(NCORES):
        out[:, docids[m]] = res.results[m]["out"]
    return np.ascontiguousarray(out)
